# revision 27
# baseline (speedup 1.0000x reference)
# Multi-head attention block (projections + softmax attention + output
# projection + residual + LayerNorm) for Trainium2, 8 NeuronCores.
#
# Sharding: data-parallel. 8 cores = 4 batches x 2 query-halves. Core c
# handles batch c//2, query rows (c%2)*1024 .. +1024. Each core receives
# the full K/V of its batch plus all weights, and produces its 1024 rows
# of the final output. No cross-core communication.
#
# v2: fp8(e4m3) DoubleRow matmuls for all projections and attn@V (2x PE
# rate at 256-contraction), 64-contraction scores from unpadded bf16
# tiles, exp split between the Scalar engine (table exp) and the Vector
# engine (Schraudolph bit-trick exp), softmax normalization deferred via
# an appended ones-column, residual+LayerNorm in fp32.
#
# Scale bookkeeping (exact powers of two, removed in-flight):
#   W* cast to fp8 as 32*W; inputs Q/K/V cast to fp8 unscaled.
#   q,k in SBUF = 32*q_true (bf16);  scores psum = 8192*s_true.
#   exp computed as exp(s - 2)  (the e^-2 guards fp8 e4m3 max 240).
#   v in SBUF = 32*v_true (fp8); o_psum = 32*o; outT8 = 64*(o/l) fp8;
#   O-proj psum = 2048*(attn_out); removed by the +residual op.

from contextlib import ExitStack

import numpy as np

import concourse.bass as bass
import concourse.mybir as mybir
import concourse.tile as tile
from concourse import bacc
from concourse.bass_utils import run_bass_kernel_spmd
from concourse.masks import make_identity

B, S, D, H = 4, 2048, 1024, 16
HD = D // H          # 64 head dim
EPS = 1e-6
NCORES = 8
SQ = (B * S) // NCORES   # 1024 query rows per core
SK = S                   # 2048 keys used in attention per core
SKL = S // 2             # 1024 keys projected locally; the other half
                         # arrives from the pair core via AllGather
P = 128

FP32 = mybir.dt.float32
BF16 = mybir.dt.bfloat16
FP8 = mybir.dt.float8e4
FP8E5 = mybir.dt.float8e5
I32 = mybir.dt.int32
I8 = mybir.dt.int8

ET = D // P     # 8  e (input-feature) tiles
DT = D // P     # 8  d (output-feature) tiles == head pairs
IT = SQ // P    # 8  query row-tiles
JT = SK // P    # 16 key row-tiles (full, in attention)
JTL = SKL // P  # 8  key row-tiles projected locally
JT2 = JT // 2   # 8  key row-tile pairs (DoubleRow planes)
# staged-exchange buffer widths (int8 bytes per partition)
KBT_B = DT * SKL * 2          # 16384: kbt local half, bf16
V8_B = (JT2 // 2) * 2 * H * 68  # 8704: v8 local half, fp8
CC_B = KBT_B + V8_B           # 25088

# Schraudolph exp on DVE, writing an e5m2 bit pattern into int8:
# bitcast_e5m2(int8(ps*SCH_A8 + SCH_B8)) ~= exp(ps/8192)  (the e5m2
# exponent bias 15 is folded into SCH_B8).  Valid for scores in
# [-10.4, +11.6]; the real data spans ~[-9.4, 9.3].
_LOG2E = 1.4426950408889634
SCH_A8 = 4.0 * _LOG2E / 8192.0
SCH_B8 = 4.0 * 15.0
EXP_SCALE = 1.0 / 8192.0
EXP_BIAS = 0.0

MULT = mybir.AluOpType.mult
ADD = mybir.AluOpType.add
SUB = mybir.AluOpType.subtract
DR = mybir.MatmulPerfMode.DoubleRow
ACT_EXP = mybir.ActivationFunctionType.Exp
ACT_COPY = mybir.ActivationFunctionType.Copy
ACT_SQRT = mybir.ActivationFunctionType.Sqrt


def _emit(tc: tile.TileContext, ctx: ExitStack):
    nc = tc.nc

    Q = nc.dram_tensor("Q", [SQ, D], FP32, kind="ExternalInput").ap()
    K = nc.dram_tensor("K", [SKL, D], FP32, kind="ExternalInput").ap()
    V = nc.dram_tensor("V", [SKL, D], FP32, kind="ExternalInput").ap()
    Wq = nc.dram_tensor("Wq", [D, D], FP32, kind="ExternalInput").ap()
    Wk = nc.dram_tensor("Wk", [D, D], FP32, kind="ExternalInput").ap()
    Wv = nc.dram_tensor("Wv", [D, D], FP32, kind="ExternalInput").ap()
    Wo = nc.dram_tensor("Wo", [D, D], FP32, kind="ExternalInput").ap()
    gamma = nc.dram_tensor("ln_gamma", [D], FP32, kind="ExternalInput").ap()
    beta = nc.dram_tensor("ln_beta", [D], FP32, kind="ExternalInput").ap()
    out = nc.dram_tensor("out", [SQ, D], FP32, kind="ExternalOutput").ap()

    persist = ctx.enter_context(tc.tile_pool(name="persist", bufs=1))
    stage = ctx.enter_context(tc.tile_pool(name="stage", bufs=4))
    cast16 = ctx.enter_context(tc.tile_pool(name="cast16", bufs=4))

    ident = persist.tile([P, P], BF16, tag="ident", name="ident")
    make_identity(nc, ident[:])

    gamma_b = persist.tile([P, D], FP32, tag="gamma_b", name="gamma_b")
    nc.gpsimd.dma_start(out=gamma_b[:], in_=gamma[None, :].to_broadcast((P, D)))
    beta_b = persist.tile([P, D], FP32, tag="beta_b", name="beta_b")
    nc.gpsimd.dma_start(out=beta_b[:], in_=beta[None, :].to_broadcast((P, D)))
    eps_t = persist.tile([P, 1], FP32, tag="eps_t", name="eps_t")
    nc.vector.memset(eps_t[:], EPS)

    # ---- persistent data tiles ----
    qT = persist.tile([P, DT, SQ], BF16, tag="qT", name="qT")      # 32*q, [d | dt, i]
    kbt = persist.tile([P, DT, SK], BF16, tag="kbt", name="kbt")   # 32*k, [d | dt, j]
    # 32*v + ones column: [j | jt2, plane u, head, 64 v + 1 ones + 3 pad]
    v8 = persist.tile([P, JT2, 2, H, 68], FP8, tag="v8", name="v8")
    # 64*(o/l): [d-of-pair | dt2, plane dt%2, i]
    outT8 = persist.tile([P, DT // 2, 2, SQ], FP8, tag="outT8", name="outT8")
    WoT8 = persist.tile([P, DT, D], FP8, tag="WoT8", name="WoT8")

    # ================= prelude: transposes + projections =================
    # Software-pipelined emission: each input row-tile is DMA'd, cast to
    # bf16 (rotating DVE/ACT/GpSimd), PE-transposed, and evac'd; projection
    # matmul chunks are interleaved between row-tiles as their operand
    # slices complete, so the PE always has dense full-array work while
    # casts/evacs catch up (also keeps HAM at K=8/8 from early on).
    pre_psum_ctx = ExitStack()
    psum_t = pre_psum_ctx.enter_context(tc.tile_pool(name="psum_t", bufs=3, space="PSUM"))
    psum_p = pre_psum_ctx.enter_context(tc.tile_pool(name="psum_p", bufs=3, space="PSUM"))
    tcount = [0]
    ecount = [0]
    pending = []  # proj-chunk closures, drained between row-tiles

    def row_tile(dram, rt, dst8, scale):
        st = stage.tile([P, D], FP32, tag="stage", name="stage")
        nc.sync.dma_start(out=st[:], in_=dram[rt * P : (rt + 1) * P, :])
        cb = cast16.tile([P, D], BF16, tag="cast16", name="cast16")
        # cast on ACT 1/3, DVE 2/3 (GpSimd is ~5x too slow for this op)
        if tcount[0] % 3 == 0:
            if scale == 1.0:
                nc.scalar.copy(out=cb[:], in_=st[:])
            else:
                nc.scalar.activation(out=cb[:], in_=st[:], func=ACT_COPY, scale=scale)
        else:
            nc.vector.tensor_scalar(
                out=cb[:], in0=st[:], scalar1=scale, scalar2=None, op0=MULT
            )
        tcount[0] += 1
        for half in range(2):
            pt = psum_t.tile([P, 512], BF16, tag="pt", name="pt")
            for k in range(4):
                et = half * 4 + k
                nc.tensor.transpose(
                    pt[:, k * P : (k + 1) * P],
                    cb[:, et * P : (et + 1) * P],
                    ident[:],
                )
            dst = dst8[:, half * 4 : half * 4 + 4, rt * P : (rt + 1) * P]
            src = pt[:].rearrange("p (k r) -> p k r", r=P)
            if ecount[0] % 2 == 0:
                nc.vector.tensor_copy(out=dst, in_=src)
            else:
                nc.scalar.copy(out=dst, in_=src)
            ecount[0] += 1

    def evac(dst, pp):
        if ecount[0] % 2 == 0:
            nc.vector.tensor_copy(out=dst, in_=pp[:])
        else:
            nc.scalar.copy(out=dst, in_=pp[:])
        ecount[0] += 1

    def kproj_chunk(WT, XT, dst, dts, csl):
        # dst[:, dt, csl] = sum_e WT[:,e,dt*P:+P].T @ XT[:,e,csl]
        # fp8 inputs, DoubleRow (256-contraction) matmuls; bf16 output
        def emit():
            for dt in dts:
                pp = psum_p.tile([P, 512], FP32, tag="pp", name="pp")
                for et2 in range(4):
                    nc.tensor.matmul(
                        pp[:],
                        WT[:, 2 * et2 : 2 * et2 + 2, dt * P : (dt + 1) * P],
                        XT[:, 2 * et2 : 2 * et2 + 2, csl],
                        start=(et2 == 0),
                        stop=(et2 == 3),
                        perf_mode=DR,
                    )
                evac(dst[:, dt, csl], pp)
        return emit

    def vproj_chunk(WvT8, VT8, jt, dc):
        def emit():
            pp = psum_p.tile([P, 512], FP32, tag="pp", name="pp")
            for et2 in range(4):
                nc.tensor.matmul(
                    pp[:],
                    VT8[:, 2 * et2 : 2 * et2 + 2, jt * P : (jt + 1) * P],
                    WvT8[:, 2 * et2 : 2 * et2 + 2, dc * 512 : (dc + 1) * 512],
                    start=(et2 == 0),
                    stop=(et2 == 3),
                    perf_mode=DR,
                )
            nc.scalar.copy(
                out=v8[:, jt // 2, jt % 2, dc * 8 : (dc + 1) * 8, 0:64],
                in_=pp[:].rearrange("p (h d) -> p h d", d=64),
            )
        return emit

    def drain(n):
        for _ in range(min(n, len(pending))):
            pending.pop(0)()

    # Each core projects only its LOCAL half of K/V (1024 keys); the pair
    # cores (2b, 2b+1) then exchange halves with a pairwise AllGather over
    # DRAM staging buffers, overlapped with the Q stage.  On odd cores the
    # two key halves land in swapped order, which is harmless: softmax and
    # attn@V both sum over all keys.
    ccs = nc.dram_tensor("cc_send", [P, CC_B], I8, kind="Internal").ap()
    ccr = nc.dram_tensor("cc_recv", [2, P, CC_B], I8, kind="Internal").ap()

    vtr_ctx = ExitStack()
    ktr_ctx = ExitStack()
    qtr_ctx = ExitStack()

    # ---- Wv + V transposes, V-proj chunks inline ----
    # ones everywhere in v8; v-proj evac overwrites data cols, leaving col
    # 64 (and pad) = 1.0 for the softmax denominator row
    nc.gpsimd.memset(v8[:], 1.0)
    vtr = vtr_ctx.enter_context(tc.tile_pool(name="vtr", bufs=1))
    WvT8 = vtr.tile([P, ET, D], FP8, tag="WvT8", name="WvT8")
    VT8 = vtr.tile([P, ET, SKL], FP8, tag="VT8", name="VT8")
    for rt in range(DT):
        row_tile(Wv, rt, WvT8, 32.0)
    for rt in range(JTL):
        if rt >= 1:
            drain(2)
        row_tile(V, rt, VT8, 1.0)
        pending.append(vproj_chunk(WvT8, VT8, rt, 0))
        pending.append(vproj_chunk(WvT8, VT8, rt, 1))
    drain(len(pending))
    vtr_ctx.close()
    nc.sync.dma_start(
        out=ccs[:, KBT_B:CC_B].rearrange("p (a u h c) -> p a u h c", a=4, u=2, h=H),
        in_=v8[:, 0 : JT2 // 2].bitcast(I8),
    )

    # ---- Wk + K transposes, K-proj chunks inline ----
    ktr = ktr_ctx.enter_context(tc.tile_pool(name="ktr", bufs=1))
    WkT8 = ktr.tile([P, ET, D], FP8, tag="WkT8", name="WkT8")
    KT8 = ktr.tile([P, ET, SKL], FP8, tag="KT8", name="KT8")
    for rt in range(DT):
        row_tile(Wk, rt, WkT8, 32.0)
    for rt in range(JTL):
        drain(1)
        row_tile(K, rt, KT8, 1.0)
        if rt % 4 == 3:
            jb = rt // 4
            csl = slice(jb * 512, (jb + 1) * 512)
            for dts in ([0, 1], [2, 3], [4, 5], [6, 7]):
                pending.append(kproj_chunk(WkT8, KT8, kbt, dts, csl))
    drain(len(pending))
    ktr_ctx.close()
    nc.sync.dma_start(
        out=ccs[:, 0:KBT_B].rearrange("p (a b) -> p a b", a=DT),
        in_=kbt[:, :, 0:SKL].bitcast(I8),
    )

    # ---- pairwise key/value-half exchange, overlapped with the Q stage ----
    nc.gpsimd.collective_compute(
        "AllGather",
        mybir.AluOpType.bypass,
        replica_groups=[[0, 1], [2, 3], [4, 5], [6, 7]],
        ins=[ccs[:, :]],
        outs=[ccr[:, :, :]],
    )

    # ---- Wq + Q transposes, Q-proj chunks inline ----
    qtr = qtr_ctx.enter_context(tc.tile_pool(name="qtr", bufs=1))
    WqT8 = qtr.tile([P, ET, D], FP8, tag="WqT8", name="WqT8")
    QT8 = qtr.tile([P, ET, SQ], FP8, tag="QT8", name="QT8")
    for rt in range(DT):
        drain(1)
        row_tile(Wq, rt, WqT8, 32.0)
    for rt in range(IT):
        drain(1)
        row_tile(Q, rt, QT8, 1.0)
        if rt % 4 == 3:
            ib = rt // 4
            csl = slice(ib * 512, (ib + 1) * 512)
            for dts in ([0, 1], [2, 3], [4, 5], [6, 7]):
                pending.append(kproj_chunk(WqT8, QT8, qT, dts, csl))

    # ---- unpack the gathered halves (both slots; order-invariant) ----
    for s_ in range(2):
        nc.sync.dma_start(
            out=kbt[:, :, s_ * SKL : (s_ + 1) * SKL],
            in_=ccr[s_, :, 0:KBT_B].bitcast(BF16).rearrange(
                "p (a b) -> p a b", a=DT
            ),
        )
        nc.sync.dma_start(
            out=v8[:, s_ * 4 : (s_ + 1) * 4],
            in_=ccr[s_, :, KBT_B:CC_B].bitcast(FP8).rearrange(
                "p (a u h c) -> p a u h c", a=4, u=2, h=H
            ),
        )

    # ---- Wo transposes; Q-proj leftovers inline ----
    for rt in range(DT):
        drain(1)
        row_tile(Wo, rt, WoT8, 32.0)
    drain(len(pending))
    qtr_ctx.close()
    pre_psum_ctx.close()

    # ========== attention + fused output tail, split over i-halves ==========
    # Head-PAIR processing via 64x128 PE row tiling: head A (= 2*dt) lives in
    # SBUF partitions 0-63, head B (= 2*dt+1) in 64-127.  Per key tile the A
    # score matmul runs on array rows 0-63 (tile T0) and the B matmul on rows
    # 64-127 (T8) CONCURRENTLY -- full array activity (keeps HAM at K=8/8)
    # and half the score matmul slots vs sequential heads.
    # The query range is processed in two 512-wide halves (ib); the O-proj +
    # residual + LayerNorm for half 0 is interleaved into half 1's attention
    # passes so the tail is mostly hidden.
    attn_ctx = ExitStack()
    expt_pool = attn_ctx.enter_context(tc.tile_pool(name="expt", bufs=6))
    norm_pool = attn_ctx.enter_context(tc.tile_pool(name="norm", bufs=3))
    ln_pool = attn_ctx.enter_context(tc.tile_pool(name="ln", bufs=2))
    psum_s = attn_ctx.enter_context(tc.tile_pool(name="psum_s", bufs=2, space="PSUM"))
    psum_o = attn_ctx.enter_context(tc.tile_pool(name="psum_o", bufs=2, space="PSUM"))
    psum_f = attn_ctx.enter_context(tc.tile_pool(name="psum_f", bufs=2, space="PSUM"))
    dram_sc = attn_ctx.enter_context(tc.tile_pool(name="dram_sc", bufs=4, space="DRAM"))

    def emit_exp(ps, ex, u, on_act):
        # exp of one [128, 512] score tile into ex[:, u, :]
        if on_act:
            nc.scalar.activation(
                out=ex[:, u, :], in_=ps[:], func=ACT_EXP, scale=EXP_SCALE,
            )
        else:
            nc.vector.tensor_scalar(
                out=ex[:, u, :].bitcast(I8), in0=ps[:],
                scalar1=SCH_A8, scalar2=SCH_B8, op0=MULT, op1=ADD,
            )

    def attn_v(po, h, jt2, ex):
        # 32*o_unnorm[d, i] (+ row 64 = softmax denom l[i])
        nc.tensor.matmul(
            po[0:65, :],
            v8[:, jt2, :, h, 0:65],
            ex[:, :, :],
            start=(jt2 == 0),
            stop=(jt2 == JT2 - 1),
            perf_mode=DR,
        )

    def norm(po, h, ib):
        # drain po to SBUF (frees the psum buffer), then normalize off the
        # SBUF copy: outT8 = pox * (1/l) = 32*(o/l)
        dt, hh = h // 2, h % 2
        isl = slice(ib * 512, (ib + 1) * 512)
        pox = norm_pool.tile([65, 512], FP32, tag="pox", name="pox")
        if hh == 0:
            nc.vector.tensor_copy(out=pox[:], in_=po[0:65, :])
        else:
            nc.scalar.copy(out=pox[:], in_=po[0:65, :])
        rd = dram_sc.tile([1, 512], FP32, tag="rd", name="rd")
        nc.sync.dma_start(out=rd[:], in_=pox[64:65, :])
        rlb = norm_pool.tile([HD, 512], FP32, tag="rlb", name="rlb")
        nc.gpsimd.dma_start(out=rlb[:], in_=rd[:].to_broadcast((HD, 512)))
        # reciprocal on 64 partitions (single-partition approx_fast mislowers)
        nc.vector.reciprocal_approx_fast(out=rlb[:], in_=rlb[:])
        # normalize multiplies on GpSimd (both exp engines are saturated)
        if hh == 0:
            nc.gpsimd.tensor_mul(
                out=outT8[0:HD, dt // 2, dt % 2, isl], in0=pox[0:HD, :], in1=rlb[:]
            )
        else:
            tmp8 = norm_pool.tile([HD, 512], FP8, tag="tmp8", name="tmp8")
            nc.gpsimd.tensor_mul(out=tmp8[:], in0=pox[0:HD, :], in1=rlb[:])
            nc.sync.dma_start(out=outT8[HD:P, dt // 2, dt % 2, isl], in_=tmp8[:])

    def pop(pend, ib):
        poA_, poB_, dt_, jt2_, exA_, exB_ = pend.pop(0)
        attn_v(poA_, 2 * dt_, jt2_, exA_)
        attn_v(poB_, 2 * dt_ + 1, jt2_, exB_)
        if jt2_ == JT2 - 1:
            norm(poA_, 2 * dt_, ib)
            norm(poB_, 2 * dt_ + 1, ib)

    def tail_mm(it):
        # O-proj + residual + bn stats for one 128-query-row tile.  Only
        # table-free ACT/DVE ops here, so it can interleave with attention
        # exp without thrashing the ACT function table.  Returns (f, mv)
        # for the deferred finale.
        rq = stage.tile([P, D], FP32, tag="stage", name="stage")
        nc.sync.dma_start(out=rq[:], in_=Q[it * P : (it + 1) * P, :])
        f = ln_pool.tile([P, D], FP32, tag="f", name="f", bufs=6)
        stats = ln_pool.tile([P, 2, 6], FP32, tag="stats", name="stats")
        for ecc in range(2):
            esl = slice(ecc * 512, (ecc + 1) * 512)
            pf = psum_f.tile([P, 512], FP32, tag="pf", name="pf")
            for dt2 in range(DT // 2):
                nc.tensor.matmul(
                    pf[:],
                    outT8[:, dt2, :, it * P : (it + 1) * P],
                    WoT8[:, 2 * dt2 : 2 * dt2 + 2, esl],
                    start=(dt2 == 0),
                    stop=(dt2 == DT // 2 - 1),
                    perf_mode=DR,
                )
            nc.vector.scalar_tensor_tensor(
                out=f[:, esl], in0=pf[:], scalar=1.0 / 1024.0,
                in1=rq[:, esl], op0=MULT, op1=ADD,
            )
            nc.vector.bn_stats(out=stats[:, ecc, :], in_=f[:, esl])
        mv = ln_pool.tile([P, 2], FP32, tag="mv", name="mv", bufs=6)
        nc.vector.bn_aggr(out=mv[:], in_=stats[:])
        return f, mv

    def tail_fin(it, f, mv, gb_gpsimd):
        # LayerNorm finale (table-based ACT ops + gamma/beta)
        rstd = ln_pool.tile([P, 1], FP32, tag="rstd", name="rstd")
        nc.scalar.activation(
            out=rstd[:], in_=mv[:, 1:2], func=ACT_SQRT, bias=eps_t[:], scale=1.0
        )
        nc.vector.reciprocal(out=rstd[:], in_=rstd[:])
        # normalize on ACT via per-partition affine: o = f*rstd - mu*rstd
        nmr = ln_pool.tile([P, 1], FP32, tag="nmr", name="nmr")
        nc.vector.tensor_scalar(
            out=nmr[:], in0=mv[:, 0:1], scalar1=rstd[:], scalar2=-1.0,
            op0=MULT, op1=MULT,
        )
        o_sb = ln_pool.tile([P, D], FP32, tag="o", name="o")
        nc.scalar.activation(
            out=o_sb[:], in_=f[:],
            func=mybir.ActivationFunctionType.Identity,
            scale=rstd[:], bias=nmr[:],
        )
        meng = nc.gpsimd if gb_gpsimd else nc.vector
        meng.tensor_mul(out=o_sb[:], in0=o_sb[:], in1=gamma_b[:])
        nc.vector.tensor_add(out=o_sb[:], in0=o_sb[:], in1=beta_b[:])
        nc.sync.dma_start(out=out[it * P : (it + 1) * P, :], in_=o_sb[:])

    def attention_pass(ib, tail_jobs):
        isl = slice(ib * 512, (ib + 1) * 512)
        pend = []
        for dt in range(DT):
            poA = psum_o.tile([P, 512], FP32, tag="po", name="poA")
            poB = psum_o.tile([P, 512], FP32, tag="po", name="poB")
            for jt2 in range(JT2):
                exA = expt_pool.tile([P, 2, 512], FP8E5, tag="ex", name="exA")
                exB = expt_pool.tile([P, 2, 512], FP8E5, tag="ex", name="exB")
                for u in range(2):
                    jt = 2 * jt2 + u
                    jsl = slice(jt * P, (jt + 1) * P)
                    # scoresT[j, i] = sum_d (32k)[j,d] (32q)[i,d], heads A+B
                    # issued back-to-back on PE row tiles T0/T8 (concurrent)
                    psA = psum_s.tile([P, 512], FP32, tag="psA", name="psA")
                    psB = psum_s.tile([P, 512], FP32, tag="psB", name="psB")
                    nc.tensor.matmul(
                        psA[:], kbt[0:HD, dt, jsl], qT[0:HD, dt, isl],
                        start=True, stop=True,
                    )
                    nc.tensor.matmul(
                        psB[:], kbt[HD:P, dt, jsl], qT[HD:P, dt, isl],
                        start=True, stop=True,
                    )
                    emit_exp(psA, exA, u, on_act=True)
                    emit_exp(psB, exB, u, on_act=False)
                pend.append((poA, poB, dt, jt2, exA, exB))
                if len(pend) > 2:
                    pop(pend, ib)
            if tail_jobs:
                tail_jobs.pop(0)()
        while pend:
            pop(pend, ib)

    attention_pass(0, [])
    # O-proj + stats for i-half 0 interleave into half 1's attention; the
    # table-based LN finales are deferred past the last exp so the ACT
    # exp table is loaded exactly once
    fin = {}
    attention_pass(1, [lambda it=it: fin.update({it: tail_mm(it)}) for it in range(4)])
    for it in range(4, IT):
        fin[it] = tail_mm(it)
        jf = it - 4
        tail_fin(jf, *fin.pop(jf), gb_gpsimd=(jf % 2 == 0))
    for it in range(4, IT):
        tail_fin(it, *fin.pop(it), gb_gpsimd=(it % 2 == 0))
    attn_ctx.close()


_CACHE = {}


def build_program():
    if "nc" not in _CACHE:
        nc = bacc.Bacc(
            "TRN2",
            target_bir_lowering=False,
            debug=False,
            enable_asserts=False,
            num_devices=NCORES,
        )
        with tile.TileContext(nc) as tc, ExitStack() as ctx:
            _emit(tc, ctx)
        nc.compile()
        _CACHE["nc"] = nc
    return _CACHE["nc"]


def shard_inputs(inputs):
    arr = {k: np.ascontiguousarray(np.asarray(v, dtype=np.float32)) for k, v in inputs.items()}
    in_maps = []
    for c in range(NCORES):
        b, hf = c // 2, c % 2
        in_maps.append(
            {
                "Q": np.ascontiguousarray(arr["Q"][b, hf * SQ : (hf + 1) * SQ, :]),
                # each core projects only its half of the keys/values; the
                # pair core's half arrives on-device via AllGather
                "K": np.ascontiguousarray(arr["K"][b, hf * SKL : (hf + 1) * SKL, :]),
                "V": np.ascontiguousarray(arr["V"][b, hf * SKL : (hf + 1) * SKL, :]),
                "Wq": arr["Wq"],
                "Wk": arr["Wk"],
                "Wv": arr["Wv"],
                "Wo": arr["Wo"],
                "ln_gamma": arr["ln_gamma"],
                "ln_beta": arr["ln_beta"],
            }
        )
    return in_maps


def unshard_outputs(results):
    full = np.zeros((B, S, D), np.float32)
    for c in range(NCORES):
        b, hf = c // 2, c % 2
        full[b, hf * SQ : (hf + 1) * SQ, :] = results[c]["out"]
    return full


def kernel(**inputs):
    nc = build_program()
    in_maps = shard_inputs(inputs)
    res = run_bass_kernel_spmd(nc, in_maps, list(range(NCORES)))
    return unshard_outputs(res.results)


if __name__ == "__main__":
    rng = np.random.default_rng(0)
    ins = {
        "Q": rng.standard_normal((B, S, D), np.float32),
        "K": rng.standard_normal((B, S, D), np.float32),
        "V": rng.standard_normal((B, S, D), np.float32),
        "Wq": rng.standard_normal((D, D), np.float32) / np.sqrt(D),
        "Wk": rng.standard_normal((D, D), np.float32) / np.sqrt(D),
        "Wv": rng.standard_normal((D, D), np.float32) / np.sqrt(D),
        "Wo": rng.standard_normal((D, D), np.float32) / np.sqrt(D),
        "ln_gamma": np.ones(D, np.float32),
        "ln_beta": np.zeros(D, np.float32),
    }
    out = kernel(**ins)
    print(out.shape, out.dtype, np.abs(out).max())



# revision 30
# speedup vs baseline: 1.1060x; 1.1060x over previous
# Multi-head attention block (projections + softmax attention + output
# projection + residual + LayerNorm) for Trainium2, 8 NeuronCores.
#
# Sharding: data-parallel. 8 cores = 4 batches x 2 query-halves. Core c
# handles batch c//2, query rows (c%2)*1024 .. +1024. Each core receives
# the full K/V of its batch plus all weights, and produces its 1024 rows
# of the final output. No cross-core communication.
#
# v2: fp8(e4m3) DoubleRow matmuls for all projections and attn@V (2x PE
# rate at 256-contraction), 64-contraction scores from unpadded bf16
# tiles, exp split between the Scalar engine (table exp) and the Vector
# engine (Schraudolph bit-trick exp), softmax normalization deferred via
# an appended ones-column, residual+LayerNorm in fp32.
#
# Scale bookkeeping (exact powers of two, removed in-flight):
#   W* cast to fp8 as 32*W; inputs Q/K/V cast to fp8 unscaled.
#   q,k in SBUF = 32*q_true (bf16);  scores psum = 8192*s_true.
#   exp computed as exp(s - 2)  (the e^-2 guards fp8 e4m3 max 240).
#   v in SBUF = 32*v_true (fp8); o_psum = 32*o; outT8 = 64*(o/l) fp8;
#   O-proj psum = 2048*(attn_out); removed by the +residual op.

from contextlib import ExitStack

import numpy as np

import concourse.bass as bass
import concourse.mybir as mybir
import concourse.tile as tile
from concourse import bacc
from concourse.bass_utils import run_bass_kernel_spmd
from concourse.masks import make_identity

B, S, D, H = 4, 2048, 1024, 16
HD = D // H          # 64 head dim
EPS = 1e-6
NCORES = 8
SQ = (B * S) // NCORES   # 1024 query rows per core
SK = S                   # 2048 keys per core
P = 128

FP32 = mybir.dt.float32
BF16 = mybir.dt.bfloat16
FP8 = mybir.dt.float8e4
FP8E5 = mybir.dt.float8e5
I32 = mybir.dt.int32
I8 = mybir.dt.int8

ET = D // P     # 8  e (input-feature) tiles
DT = D // P     # 8  d (output-feature) tiles == head pairs
IT = SQ // P    # 8  query row-tiles
JT = SK // P    # 16 key row-tiles
JT2 = JT // 2   # 8  key row-tile pairs (DoubleRow planes)

# Schraudolph exp on DVE, writing an e5m2 bit pattern into int8:
# bitcast_e5m2(int8(ps*SCH_A8 + SCH_B8)) ~= exp(ps/8192)  (the e5m2
# exponent bias 15 is folded into SCH_B8).  Valid for scores in
# [-10.4, +11.6]; the real data spans ~[-9.4, 9.3].
_LOG2E = 1.4426950408889634
SCH_A8 = 4.0 * _LOG2E / 8192.0
SCH_B8 = 4.0 * 15.0
EXP_SCALE = 1.0 / 8192.0
EXP_BIAS = 0.0

MULT = mybir.AluOpType.mult
ADD = mybir.AluOpType.add
SUB = mybir.AluOpType.subtract
DR = mybir.MatmulPerfMode.DoubleRow
ACT_EXP = mybir.ActivationFunctionType.Exp
ACT_COPY = mybir.ActivationFunctionType.Copy
ACT_SQRT = mybir.ActivationFunctionType.Sqrt


def _emit(tc: tile.TileContext, ctx: ExitStack):
    nc = tc.nc

    Q = nc.dram_tensor("Q", [SQ, D], FP32, kind="ExternalInput").ap()
    K = nc.dram_tensor("K", [SK, D], FP32, kind="ExternalInput").ap()
    V = nc.dram_tensor("V", [SK, D], FP32, kind="ExternalInput").ap()
    Wq = nc.dram_tensor("Wq", [D, D], FP32, kind="ExternalInput").ap()
    Wk = nc.dram_tensor("Wk", [D, D], FP32, kind="ExternalInput").ap()
    Wv = nc.dram_tensor("Wv", [D, D], FP32, kind="ExternalInput").ap()
    Wo = nc.dram_tensor("Wo", [D, D], FP32, kind="ExternalInput").ap()
    gamma = nc.dram_tensor("ln_gamma", [D], FP32, kind="ExternalInput").ap()
    beta = nc.dram_tensor("ln_beta", [D], FP32, kind="ExternalInput").ap()
    out = nc.dram_tensor("out", [SQ, D], FP32, kind="ExternalOutput").ap()

    persist = ctx.enter_context(tc.tile_pool(name="persist", bufs=1))
    stage = ctx.enter_context(tc.tile_pool(name="stage", bufs=4))
    cast16 = ctx.enter_context(tc.tile_pool(name="cast16", bufs=4))

    ident = persist.tile([P, P], BF16, tag="ident", name="ident")
    make_identity(nc, ident[:])

    gamma_b = persist.tile([P, D], FP32, tag="gamma_b", name="gamma_b")
    nc.gpsimd.dma_start(out=gamma_b[:], in_=gamma[None, :].to_broadcast((P, D)))
    beta_b = persist.tile([P, D], FP32, tag="beta_b", name="beta_b")
    nc.gpsimd.dma_start(out=beta_b[:], in_=beta[None, :].to_broadcast((P, D)))
    eps_t = persist.tile([P, 1], FP32, tag="eps_t", name="eps_t")
    nc.vector.memset(eps_t[:], EPS)

    # ---- persistent data tiles ----
    qT = persist.tile([P, DT, SQ], BF16, tag="qT", name="qT")      # 32*q, [d | dt, i]
    kbt = persist.tile([P, DT, SK], BF16, tag="kbt", name="kbt")   # 32*k, [d | dt, j]
    # 32*v + ones column: [j | jt2, plane u, head, 64 v + 1 ones + 3 pad]
    v8 = persist.tile([P, JT2, 2, H, 68], FP8, tag="v8", name="v8")
    # 64*(o/l): [d-of-pair | dt2, plane dt%2, i]
    outT8 = persist.tile([P, DT // 2, 2, SQ], FP8, tag="outT8", name="outT8")
    WoT8 = persist.tile([P, DT, D], FP8, tag="WoT8", name="WoT8")

    # ================= prelude: transposes + projections =================
    # Software-pipelined emission: each input row-tile is DMA'd, cast to
    # bf16 (rotating DVE/ACT/GpSimd), PE-transposed, and evac'd; projection
    # matmul chunks are interleaved between row-tiles as their operand
    # slices complete, so the PE always has dense full-array work while
    # casts/evacs catch up (also keeps HAM at K=8/8 from early on).
    pre_psum_ctx = ExitStack()
    psum_t = pre_psum_ctx.enter_context(tc.tile_pool(name="psum_t", bufs=3, space="PSUM"))
    psum_p = pre_psum_ctx.enter_context(tc.tile_pool(name="psum_p", bufs=3, space="PSUM"))
    tcount = [0]
    ecount = [0]
    pending = []  # proj-chunk closures, drained between row-tiles

    def row_tile(dram, rt, dst8, scale):
        st = stage.tile([P, D], FP32, tag="stage", name="stage")
        nc.sync.dma_start(out=st[:], in_=dram[rt * P : (rt + 1) * P, :])
        cb = cast16.tile([P, D], BF16, tag="cast16", name="cast16")
        # cast on ACT 1/3, DVE 2/3 (GpSimd is ~5x too slow for this op)
        if tcount[0] % 3 == 0:
            if scale == 1.0:
                nc.scalar.copy(out=cb[:], in_=st[:])
            else:
                nc.scalar.activation(out=cb[:], in_=st[:], func=ACT_COPY, scale=scale)
        else:
            nc.vector.tensor_scalar(
                out=cb[:], in0=st[:], scalar1=scale, scalar2=None, op0=MULT
            )
        tcount[0] += 1
        for half in range(2):
            pt = psum_t.tile([P, 512], BF16, tag="pt", name="pt")
            for k in range(4):
                et = half * 4 + k
                nc.tensor.transpose(
                    pt[:, k * P : (k + 1) * P],
                    cb[:, et * P : (et + 1) * P],
                    ident[:],
                )
            dst = dst8[:, half * 4 : half * 4 + 4, rt * P : (rt + 1) * P]
            src = pt[:].rearrange("p (k r) -> p k r", r=P)
            if ecount[0] % 2 == 0:
                nc.vector.tensor_copy(out=dst, in_=src)
            else:
                nc.scalar.copy(out=dst, in_=src)
            ecount[0] += 1

    def evac(dst, pp):
        if ecount[0] % 2 == 0:
            nc.vector.tensor_copy(out=dst, in_=pp[:])
        else:
            nc.scalar.copy(out=dst, in_=pp[:])
        ecount[0] += 1

    def kproj_chunk(WT, XT, dst, dts, csl):
        # dst[:, dt, csl] = sum_e WT[:,e,dt*P:+P].T @ XT[:,e,csl]
        # fp8 inputs, DoubleRow (256-contraction) matmuls; bf16 output
        def emit():
            for dt in dts:
                pp = psum_p.tile([P, 512], FP32, tag="pp", name="pp")
                for et2 in range(4):
                    nc.tensor.matmul(
                        pp[:],
                        WT[:, 2 * et2 : 2 * et2 + 2, dt * P : (dt + 1) * P],
                        XT[:, 2 * et2 : 2 * et2 + 2, csl],
                        start=(et2 == 0),
                        stop=(et2 == 3),
                        perf_mode=DR,
                    )
                evac(dst[:, dt, csl], pp)
        return emit

    def vproj_chunk(WvT8, VT8, jt, dc):
        def emit():
            pp = psum_p.tile([P, 512], FP32, tag="pp", name="pp")
            for et2 in range(4):
                nc.tensor.matmul(
                    pp[:],
                    VT8[:, 2 * et2 : 2 * et2 + 2, jt * P : (jt + 1) * P],
                    WvT8[:, 2 * et2 : 2 * et2 + 2, dc * 512 : (dc + 1) * 512],
                    start=(et2 == 0),
                    stop=(et2 == 3),
                    perf_mode=DR,
                )
            nc.scalar.copy(
                out=v8[:, jt // 2, jt % 2, dc * 8 : (dc + 1) * 8, 0:64],
                in_=pp[:].rearrange("p (h d) -> p h d", d=64),
            )
        return emit

    def drain(n):
        for _ in range(min(n, len(pending))):
            pending.pop(0)()

    ktr_ctx = ExitStack()
    qtr_ctx = ExitStack()
    vtr_ctx = ExitStack()
    ktr = ktr_ctx.enter_context(tc.tile_pool(name="ktr", bufs=1))
    WkT16 = ktr.tile([P, ET, D], FP8, tag="WkT16", name="WkT16")
    KT16 = ktr.tile([P, ET, SK], FP8, tag="KT16", name="KT16")

    # ---- Wk + K transposes, K-proj chunks inline ----
    for rt in range(DT):
        row_tile(Wk, rt, WkT16, 32.0)
    for rt in range(JT):
        drain(1)
        row_tile(K, rt, KT16, 1.0)
        if rt % 4 == 3:
            jb = rt // 4
            csl = slice(jb * 512, (jb + 1) * 512)
            for dts in ([0, 1], [2, 3], [4, 5], [6, 7]):
                pending.append(kproj_chunk(WkT16, KT16, kbt, dts, csl))

    # ---- Wq + Q transposes; leftover K-proj + Q-proj chunks inline ----
    qtr = qtr_ctx.enter_context(tc.tile_pool(name="qtr", bufs=1, side="right"))
    WqT16 = qtr.tile([P, ET, D], FP8, tag="WqT16", name="WqT16")
    QT16 = qtr.tile([P, ET, SQ], FP8, tag="QT16", name="QT16")
    for rt in range(DT):
        drain(1)
        row_tile(Wq, rt, WqT16, 32.0)
    for rt in range(IT):
        drain(1)
        row_tile(Q, rt, QT16, 1.0)
        if rt % 4 == 3:
            ib = rt // 4
            csl = slice(ib * 512, (ib + 1) * 512)
            for dts in ([0, 1], [2, 3], [4, 5], [6, 7]):
                pending.append(kproj_chunk(WqT16, QT16, qT, dts, csl))
    # all 16 K-proj chunks have drained by Q rt3 (12 in the K loop, 4 in
    # the Wq loop), so KT16/WkT16 are dead here
    ktr_ctx.close()

    # ---- Wv + V transposes; Q-proj leftovers + V-proj chunks inline ----
    # ones everywhere in v8; v-proj evac overwrites data cols, leaving col
    # 64 (and pad) = 1.0 for the softmax denominator row
    nc.gpsimd.memset(v8[:], 1.0)
    vtr = vtr_ctx.enter_context(tc.tile_pool(name="vtr", bufs=1))
    WvT8 = vtr.tile([P, ET, D], FP8, tag="WvT8", name="WvT8")
    VT8 = vtr.tile([P, ET, SK], FP8, tag="VT8", name="VT8")
    for rt in range(DT):
        drain(1)
        row_tile(Wv, rt, WvT8, 32.0)
    drain(len(pending))  # flush Q-proj before QT16's pool closes
    qtr_ctx.close()
    for rt in range(JT):
        if rt >= 1:
            drain(2)
        row_tile(V, rt, VT8, 1.0)
        pending.append(vproj_chunk(WvT8, VT8, rt, 0))
        pending.append(vproj_chunk(WvT8, VT8, rt, 1))

    # ---- Wo transposes; V-proj leftovers inline ----
    for rt in range(DT):
        drain(1)
        row_tile(Wo, rt, WoT8, 32.0)
    drain(len(pending))
    vtr_ctx.close()
    pre_psum_ctx.close()

    # ========== attention + fused output tail, split over i-halves ==========
    # Head-PAIR processing via 64x128 PE row tiling: head A (= 2*dt) lives in
    # SBUF partitions 0-63, head B (= 2*dt+1) in 64-127.  Per key tile the A
    # score matmul runs on array rows 0-63 (tile T0) and the B matmul on rows
    # 64-127 (T8) CONCURRENTLY -- full array activity (keeps HAM at K=8/8)
    # and half the score matmul slots vs sequential heads.
    # The query range is processed in two 512-wide halves (ib); the O-proj +
    # residual + LayerNorm for half 0 is interleaved into half 1's attention
    # passes so the tail is mostly hidden.
    attn_ctx = ExitStack()
    expt_pool = attn_ctx.enter_context(tc.tile_pool(name="expt", bufs=6))
    norm_pool = attn_ctx.enter_context(tc.tile_pool(name="norm", bufs=3))
    ln_pool = attn_ctx.enter_context(tc.tile_pool(name="ln", bufs=2))
    psum_s = attn_ctx.enter_context(tc.tile_pool(name="psum_s", bufs=2, space="PSUM"))
    psum_o = attn_ctx.enter_context(tc.tile_pool(name="psum_o", bufs=2, space="PSUM"))
    psum_f = attn_ctx.enter_context(tc.tile_pool(name="psum_f", bufs=2, space="PSUM"))
    dram_sc = attn_ctx.enter_context(tc.tile_pool(name="dram_sc", bufs=4, space="DRAM"))

    def emit_exp(ps, ex, u, on_act):
        # exp of one [128, 512] score tile into ex[:, u, :]
        if on_act:
            nc.scalar.activation(
                out=ex[:, u, :], in_=ps[:], func=ACT_EXP, scale=EXP_SCALE,
            )
        else:
            nc.vector.tensor_scalar(
                out=ex[:, u, :].bitcast(I8), in0=ps[:],
                scalar1=SCH_A8, scalar2=SCH_B8, op0=MULT, op1=ADD,
            )

    def attn_v(po, h, jt2, ex):
        # 32*o_unnorm[d, i] (+ row 64 = softmax denom l[i])
        nc.tensor.matmul(
            po[0:65, :],
            v8[:, jt2, :, h, 0:65],
            ex[:, :, :],
            start=(jt2 == 0),
            stop=(jt2 == JT2 - 1),
            perf_mode=DR,
        )

    def norm(po, h, ib):
        # drain po to SBUF (frees the psum buffer), then normalize off the
        # SBUF copy: outT8 = pox * (1/l) = 32*(o/l)
        dt, hh = h // 2, h % 2
        isl = slice(ib * 512, (ib + 1) * 512)
        pox = norm_pool.tile([65, 512], FP32, tag="pox", name="pox")
        if hh == 0:
            nc.vector.tensor_copy(out=pox[:], in_=po[0:65, :])
        else:
            nc.scalar.copy(out=pox[:], in_=po[0:65, :])
        rd = dram_sc.tile([1, 512], FP32, tag="rd", name="rd")
        nc.sync.dma_start(out=rd[:], in_=pox[64:65, :])
        rlb = norm_pool.tile([HD, 512], FP32, tag="rlb", name="rlb")
        nc.gpsimd.dma_start(out=rlb[:], in_=rd[:].to_broadcast((HD, 512)))
        # reciprocal on 64 partitions (single-partition approx_fast mislowers)
        nc.vector.reciprocal_approx_fast(out=rlb[:], in_=rlb[:])
        # normalize multiplies on GpSimd (both exp engines are saturated)
        if hh == 0:
            nc.gpsimd.tensor_mul(
                out=outT8[0:HD, dt // 2, dt % 2, isl], in0=pox[0:HD, :], in1=rlb[:]
            )
        else:
            tmp8 = norm_pool.tile([HD, 512], FP8, tag="tmp8", name="tmp8")
            nc.gpsimd.tensor_mul(out=tmp8[:], in0=pox[0:HD, :], in1=rlb[:])
            nc.sync.dma_start(out=outT8[HD:P, dt // 2, dt % 2, isl], in_=tmp8[:])

    def pop(pend, ib):
        poA_, poB_, dt_, jt2_, exA_, exB_ = pend.pop(0)
        attn_v(poA_, 2 * dt_, jt2_, exA_)
        attn_v(poB_, 2 * dt_ + 1, jt2_, exB_)
        if jt2_ == JT2 - 1:
            norm(poA_, 2 * dt_, ib)
            norm(poB_, 2 * dt_ + 1, ib)

    def tail_mm(it):
        # O-proj + residual + bn stats for one 128-query-row tile.  Only
        # table-free ACT/DVE ops here, so it can interleave with attention
        # exp without thrashing the ACT function table.  Returns (f, mv)
        # for the deferred finale.
        rq = stage.tile([P, D], FP32, tag="stage", name="stage")
        nc.sync.dma_start(out=rq[:], in_=Q[it * P : (it + 1) * P, :])
        f = ln_pool.tile([P, D], FP32, tag="f", name="f", bufs=6)
        stats = ln_pool.tile([P, 2, 6], FP32, tag="stats", name="stats")
        for ecc in range(2):
            esl = slice(ecc * 512, (ecc + 1) * 512)
            pf = psum_f.tile([P, 512], FP32, tag="pf", name="pf")
            for dt2 in range(DT // 2):
                nc.tensor.matmul(
                    pf[:],
                    outT8[:, dt2, :, it * P : (it + 1) * P],
                    WoT8[:, 2 * dt2 : 2 * dt2 + 2, esl],
                    start=(dt2 == 0),
                    stop=(dt2 == DT // 2 - 1),
                    perf_mode=DR,
                )
            nc.vector.scalar_tensor_tensor(
                out=f[:, esl], in0=pf[:], scalar=1.0 / 1024.0,
                in1=rq[:, esl], op0=MULT, op1=ADD,
            )
            nc.vector.bn_stats(out=stats[:, ecc, :], in_=f[:, esl])
        mv = ln_pool.tile([P, 2], FP32, tag="mv", name="mv", bufs=6)
        nc.vector.bn_aggr(out=mv[:], in_=stats[:])
        return f, mv

    def tail_fin(it, f, mv, gb_gpsimd):
        # LayerNorm finale (table-based ACT ops + gamma/beta)
        rstd = ln_pool.tile([P, 1], FP32, tag="rstd", name="rstd")
        nc.scalar.activation(
            out=rstd[:], in_=mv[:, 1:2], func=ACT_SQRT, bias=eps_t[:], scale=1.0
        )
        nc.vector.reciprocal(out=rstd[:], in_=rstd[:])
        # normalize on ACT via per-partition affine: o = f*rstd - mu*rstd
        nmr = ln_pool.tile([P, 1], FP32, tag="nmr", name="nmr")
        nc.vector.tensor_scalar(
            out=nmr[:], in0=mv[:, 0:1], scalar1=rstd[:], scalar2=-1.0,
            op0=MULT, op1=MULT,
        )
        o_sb = ln_pool.tile([P, D], FP32, tag="o", name="o")
        nc.scalar.activation(
            out=o_sb[:], in_=f[:],
            func=mybir.ActivationFunctionType.Identity,
            scale=rstd[:], bias=nmr[:],
        )
        meng = nc.gpsimd if gb_gpsimd else nc.vector
        meng.tensor_mul(out=o_sb[:], in0=o_sb[:], in1=gamma_b[:])
        nc.vector.tensor_add(out=o_sb[:], in0=o_sb[:], in1=beta_b[:])
        nc.sync.dma_start(out=out[it * P : (it + 1) * P, :], in_=o_sb[:])

    def attention_pass(ib, tail_jobs):
        isl = slice(ib * 512, (ib + 1) * 512)
        pend = []
        for dt in range(DT):
            poA = psum_o.tile([P, 512], FP32, tag="po", name="poA")
            poB = psum_o.tile([P, 512], FP32, tag="po", name="poB")
            for jt2 in range(JT2):
                exA = expt_pool.tile([P, 2, 512], FP8E5, tag="ex", name="exA")
                exB = expt_pool.tile([P, 2, 512], FP8E5, tag="ex", name="exB")
                for u in range(2):
                    jt = 2 * jt2 + u
                    jsl = slice(jt * P, (jt + 1) * P)
                    # scoresT[j, i] = sum_d (32k)[j,d] (32q)[i,d], heads A+B
                    # issued back-to-back on PE row tiles T0/T8 (concurrent)
                    psA = psum_s.tile([P, 512], FP32, tag="psA", name="psA")
                    psB = psum_s.tile([P, 512], FP32, tag="psB", name="psB")
                    nc.tensor.matmul(
                        psA[:], kbt[0:HD, dt, jsl], qT[0:HD, dt, isl],
                        start=True, stop=True,
                    )
                    nc.tensor.matmul(
                        psB[:], kbt[HD:P, dt, jsl], qT[HD:P, dt, isl],
                        start=True, stop=True,
                    )
                    emit_exp(psA, exA, u, on_act=True)
                    emit_exp(psB, exB, u, on_act=False)
                pend.append((poA, poB, dt, jt2, exA, exB))
                if len(pend) > 2:
                    pop(pend, ib)
            if tail_jobs:
                tail_jobs.pop(0)()
        while pend:
            pop(pend, ib)

    attention_pass(0, [])
    # O-proj + stats for i-half 0 interleave into half 1's attention; the
    # table-based LN finales are deferred past the last exp so the ACT
    # exp table is loaded exactly once
    fin = {}
    attention_pass(1, [lambda it=it: fin.update({it: tail_mm(it)}) for it in range(4)])
    for it in range(4, IT):
        fin[it] = tail_mm(it)
        jf = it - 4
        tail_fin(jf, *fin.pop(jf), gb_gpsimd=(jf % 2 == 0))
    for it in range(4, IT):
        tail_fin(it, *fin.pop(it), gb_gpsimd=(it % 2 == 0))
    attn_ctx.close()


_CACHE = {}


def build_program():
    if "nc" not in _CACHE:
        nc = bacc.Bacc(
            "TRN2",
            target_bir_lowering=False,
            debug=False,
            enable_asserts=False,
            num_devices=NCORES,
        )
        with tile.TileContext(nc) as tc, ExitStack() as ctx:
            _emit(tc, ctx)
        nc.compile()
        _CACHE["nc"] = nc
    return _CACHE["nc"]


def shard_inputs(inputs):
    arr = {k: np.ascontiguousarray(np.asarray(v, dtype=np.float32)) for k, v in inputs.items()}
    in_maps = []
    for c in range(NCORES):
        b, hf = c // 2, c % 2
        in_maps.append(
            {
                "Q": np.ascontiguousarray(arr["Q"][b, hf * SQ : (hf + 1) * SQ, :]),
                "K": arr["K"][b],
                "V": arr["V"][b],
                "Wq": arr["Wq"],
                "Wk": arr["Wk"],
                "Wv": arr["Wv"],
                "Wo": arr["Wo"],
                "ln_gamma": arr["ln_gamma"],
                "ln_beta": arr["ln_beta"],
            }
        )
    return in_maps


def unshard_outputs(results):
    full = np.zeros((B, S, D), np.float32)
    for c in range(NCORES):
        b, hf = c // 2, c % 2
        full[b, hf * SQ : (hf + 1) * SQ, :] = results[c]["out"]
    return full


def kernel(**inputs):
    nc = build_program()
    in_maps = shard_inputs(inputs)
    res = run_bass_kernel_spmd(nc, in_maps, list(range(NCORES)))
    return unshard_outputs(res.results)


if __name__ == "__main__":
    rng = np.random.default_rng(0)
    ins = {
        "Q": rng.standard_normal((B, S, D), np.float32),
        "K": rng.standard_normal((B, S, D), np.float32),
        "V": rng.standard_normal((B, S, D), np.float32),
        "Wq": rng.standard_normal((D, D), np.float32) / np.sqrt(D),
        "Wk": rng.standard_normal((D, D), np.float32) / np.sqrt(D),
        "Wv": rng.standard_normal((D, D), np.float32) / np.sqrt(D),
        "Wo": rng.standard_normal((D, D), np.float32) / np.sqrt(D),
        "ln_gamma": np.ones(D, np.float32),
        "ln_beta": np.zeros(D, np.float32),
    }
    out = kernel(**ins)
    print(out.shape, out.dtype, np.abs(out).max())

